# revision 1
# baseline (speedup 1.0000x reference)
"""TRN2 Bass kernel for nn_HCSMoEQwen3MoeSparseMoeBlock (8-core expert-parallel).

Sharding: core g owns group g's dominant expert and processes ALL tokens;
router replicated (each core computes only its group's combined weight
w_g[t]); host sums the 8 partial outputs w_g[t] * y_g[t, :].

Single software-pipelined loop over 128-token chunks; float32r matmuls
(full PE rate, ~2e-4 rel err); router logits in exact fp32 (separate
F32-typed tiles — the PE precision mode follows the backing tensor dtype):
  router: logitsT = gwT.T-stationary @ x-chunk (fp32) -> PE transpose
  M1 b-major: h_b = xT_c.T @ gu_b, 16 same-bank MMs per 512-col block
              (host-interleaved [256 gate|256 up]) -> silu+mult drains bank
  actT = PE-transpose(act);  y = actT.T @ dnT;  top-8 chain on DVE
  (pinned after casts);  out = w*y -> DRAM
"""
import numpy as np

import concourse.bass as bass
import concourse.mybir as mybir
import concourse.tile as tile
from concourse import bacc
from concourse.bass_utils import run_bass_kernel_spmd
from concourse.masks import make_identity

T = 2048
H = 2048
I2 = 1536
I = 768
E = 32
G = 8
TOP_K = 8
KO = H // 128
JO = I // 128
TCH = 128
NCHUNK = T // TCH
HB = 512
NEG_BIG = -1.0e9

F32 = mybir.dt.float32
F32R = mybir.dt.float32r
U8 = mybir.dt.uint8
AX = mybir.AxisListType.X
OP = mybir.AluOpType
ACTF = mybir.ActivationFunctionType

_CACHED_NC = None


def _build():
    global _CACHED_NC
    if _CACHED_NC is not None:
        return _CACHED_NC
    nc = bacc.Bacc("TRN2", target_bir_lowering=False, debug=False, num_devices=G)

    xT_d = nc.dram_tensor("xT", [H, T], F32R, kind="ExternalInput")
    gu_d = nc.dram_tensor("gu", [H, I2], F32R, kind="ExternalInput")
    gw_d = nc.dram_tensor("gw", [H, E], F32, kind="ExternalInput")
    dnT_d = nc.dram_tensor("dnT", [I, H], F32R, kind="ExternalInput")
    mgb_d = nc.dram_tensor("mgb", [128, E], F32, kind="ExternalInput")
    y_d = nc.dram_tensor("y", [T, H], F32, kind="ExternalOutput")

    xT_ap = xT_d.ap().rearrange("(ko p) t -> p ko t", p=128)
    xT_ap32 = xT_d.ap().bitcast(F32).rearrange("(ko p) t -> p ko t", p=128)
    gu_ap = gu_d.ap().rearrange("(ko p) o -> p ko o", p=128)
    gw_ap = gw_d.ap().rearrange("(ko p) e -> p ko e", p=128)
    dnT_ap = dnT_d.ap().rearrange("(jo p) h -> p jo h", p=128)

    with tile.TileContext(nc) as tc:
        with (
            tc.tile_pool(name="const", bufs=1) as cpool,
            tc.tile_pool(name="weights", bufs=1) as wpool,
            tc.tile_pool(name="xin", bufs=2) as xpool,
            tc.tile_pool(name="xrin", bufs=2) as xrpool,
            tc.tile_pool(name="acts", bufs=1) as apool,
            tc.tile_pool(name="router", bufs=2) as rpool,
            tc.tile_pool(name="yout", bufs=2) as ypool,
            tc.tile_pool(name="plg", bufs=1, space="PSUM") as plg,
            tc.tile_pool(name="ph", bufs=3, space="PSUM") as pph,
            tc.tile_pool(name="ps", bufs=1, space="PSUM") as pps,
            tc.tile_pool(name="py", bufs=2, space="PSUM") as ppy,
        ):
            identity = cpool.tile([128, 128], F32, tag="identity")
            make_identity(nc, identity)
            negbig = cpool.tile([128, E], F32, tag="negbig")
            nc.vector.memset(negbig, NEG_BIG)
            mgb_sb = cpool.tile([128, E], F32, tag="mgb")
            nc.sync.dma_start(mgb_sb[:], mgb_d.ap())
            gw_sb = cpool.tile([128, KO, E], F32, tag="gw")
            nc.sync.dma_start(gw_sb[:], gw_ap)

            gu_sb = wpool.tile([128, KO, I2], F32R, tag="gu")
            dn_sb = wpool.tile([128, JO, H], F32R, tag="dn")

            xtiles = {}
            xrtiles = {}

            def load_chunk(ci):
                t = xpool.tile([128, KO, TCH], F32R, tag="xT_c",
                               name=f"xT_c{ci}")
                nc.sync.dma_start(t[:], xT_ap[:, :, ci * TCH:(ci + 1) * TCH])
                xtiles[ci] = t

            def load_xr(ci):
                t = xrpool.tile([128, KO, TCH], F32, tag="xr",
                                name=f"xr{ci}")
                nc.sync.dma_start(t[:], xT_ap32[:, :, ci * TCH:(ci + 1) * TCH])
                xrtiles[ci] = t

            load_xr(0)
            load_chunk(0)
            nc.sync.dma_start(gu_sb[:, :, 0:HB], gu_ap[:, :, 0:HB])

            for tci in range(NCHUNK):
                tsl = slice(tci * TCH, (tci + 1) * TCH)
                if tci + 1 < NCHUNK:
                    load_xr(tci + 1)
                    load_chunk(tci + 1)
                xT_c = xtiles.pop(tci)
                xr_c = xrtiles.pop(tci)

                # ---- router logits (exact fp32), gw stationary ----
                lg_ps = plg.tile([E, TCH], F32, tag="lg_ps")
                for k in range(KO):
                    nc.tensor.matmul(
                        lg_ps[:], gw_sb[:, k], xr_c[:, k],
                        start=(k == 0), stop=(k == KO - 1),
                    )
                lgT_sb = rpool.tile([E, TCH], F32, tag="lgT_sb")
                nc.vector.tensor_copy(lgT_sb[:], lg_ps[:])
                s_ps = pps.tile([128, JO + 1, TCH], F32, tag="s_ps")
                nc.tensor.transpose(s_ps[:, JO, :E], lgT_sb[:],
                                    identity[:E, :E])
                logits = rpool.tile([128, E], F32, tag="logits")
                nc.vector.tensor_copy(logits[:], s_ps[:, JO, :E])

                if tci == 0:
                    nc.sync.dma_start(gu_sb[:, :, HB:2 * HB],
                                      gu_ap[:, :, HB:2 * HB])

                # ---- M1, b-major: one PSUM bank at a time ----
                act_sb = apool.tile([128, I], F32, tag="act")
                for b in range(3):
                    h_ps = pph.tile([128, HB], F32, tag="h_ps",
                                    name=f"h{tci}_{b}")
                    for k in range(KO):
                        nc.tensor.matmul(
                            h_ps[:], xT_c[:, k],
                            gu_sb[:, k, b * HB:(b + 1) * HB],
                            start=(k == 0), stop=(k == KO - 1),
                        )
                    if tci == 0 and b == 0:
                        nc.sync.dma_start(gu_sb[:, :, 2 * HB:I2],
                                          gu_ap[:, :, 2 * HB:I2])
                    if tci == 0 and b == 1:
                        for j in range(JO):
                            nc.sync.dma_start(dn_sb[:, j], dnT_ap[:, j])
                    # host interleave: h_b = [256 gate | 256 up]
                    silu_sb = apool.tile([128, 256], F32, tag="silu")
                    nc.scalar.activation(silu_sb[:], h_ps[:, :256],
                                         ACTF.Silu)
                    nc.vector.tensor_tensor(
                        act_sb[:, 256 * b:256 * (b + 1)], silu_sb[:],
                        h_ps[:, 256:], OP.mult,
                    )

                # ---- transpose act -> actT ----
                actT_sb = apool.tile([128, JO, TCH], F32R, tag="actT")
                for j in range(JO):
                    nc.tensor.transpose(
                        s_ps[:, j], act_sb[:, j * 128:(j + 1) * 128],
                        identity,
                    )
                    nc.vector.tensor_copy(actT_sb[:, j], s_ps[:, j])

                # ---- M2 ----
                y_pss = []
                for hb in range(H // HB):
                    y_ps = ppy.tile([128, HB], F32, tag="y_ps",
                                    name=f"y_ps{tci}_{hb}")
                    for j in range(JO):
                        nc.tensor.matmul(
                            y_ps[:], actT_sb[:, j],
                            dn_sb[:, j, hb * HB:(hb + 1) * HB],
                            start=(j == 0), stop=(j == JO - 1),
                        )
                    y_pss.append(y_ps)

                # ---- top-8 router chain (DVE) ----
                # Pin the chain after the casts so it can't hog DVE while the
                # next chunk's SwiGLU needs the h banks released. The first
                # chunk has an idle DVE (DMA-bound head) and the last has no
                # successor to protect, so let those chains run early.
                cur = rpool.tile([128, E], F32, tag="cur")
                if 0 < tci < NCHUNK - 1:
                    dep = rpool.tile([128, E], F32, tag="dep")
                    nc.vector.tensor_scalar(
                        dep[:], actT_sb[:, JO - 1, :E].bitcast(F32), 0.0,
                        None, OP.mult)
                    nc.vector.tensor_tensor(cur[:], logits[:], dep[:], OP.add)
                else:
                    nc.vector.tensor_copy(cur[:], logits[:])
                msk = rpool.tile([128, E], U8, tag="msk")
                m1 = rpool.tile([128, 1], F32, tag="m1")
                mk = rpool.tile([128, 1], F32, tag="mk")
                for it in range(TOP_K - 1):
                    tgt = m1 if it == 0 else mk
                    nc.vector.reduce_max(tgt[:], cur[:], axis=AX)
                    nc.vector.tensor_scalar(msk[:], cur[:], tgt[:],
                                            None, OP.is_ge)
                    nc.vector.copy_predicated(cur[:], msk[:], negbig[:])
                m8 = rpool.tile([128, 1], F32, tag="m8")
                nc.vector.reduce_max(m8[:], cur[:], axis=AX)

                nm1 = rpool.tile([128, 1], F32, tag="nm1")
                nc.vector.tensor_scalar(nm1[:], m1[:], -1.0, None, OP.mult)
                mask8 = rpool.tile([128, E], F32, tag="mask8")
                nc.vector.tensor_scalar(mask8[:], logits[:], m8[:],
                                        None, OP.is_ge)
                ew = rpool.tile([128, E], F32, tag="ew")
                nc.scalar.activation(ew[:], logits[:], ACTF.Exp, bias=nm1[:])
                nc.vector.tensor_tensor(ew[:], ew[:], mask8[:], OP.mult)
                s8 = rpool.tile([128, 1], F32, tag="s8")
                nc.vector.reduce_sum(s8[:], ew[:], axis=AX)
                nc.vector.tensor_tensor(ew[:], ew[:], mgb_sb[:], OP.mult)
                num = rpool.tile([128, 1], F32, tag="num")
                nc.vector.reduce_sum(num[:], ew[:], axis=AX)
                rs = rpool.tile([128, 1], F32, tag="rs")
                nc.vector.reciprocal(rs[:], s8[:])
                w_t = rpool.tile([128, 1], F32, tag="w_t")
                nc.vector.tensor_tensor(w_t[:], num[:], rs[:], OP.mult)

                # ---- scale + store ----
                for hb in range(H // HB):
                    y_sb = ypool.tile([128, HB], F32, tag="y_sb")
                    nc.vector.tensor_scalar(
                        y_sb[:], y_pss[hb][:], w_t[:], None, OP.mult,
                    )
                    nc.sync.dma_start(
                        y_d.ap()[tsl, hb * HB:(hb + 1) * HB], y_sb[:],
                    )
    nc.compile()
    _CACHED_NC = nc
    return nc


_GATEUP_PERM = np.concatenate(
    [np.r_[256 * b:256 * b + 256, 768 + 256 * b:768 + 256 * b + 256]
     for b in range(3)]
)


def prepare_in_maps(hidden_states, gate_weight, gate_up_proj, down_proj,
                    merge_groups, dominant_experts):
    x = np.asarray(hidden_states, dtype=np.float32).reshape(T, H)
    xT = np.ascontiguousarray(x.T)
    gw = np.asarray(gate_weight, dtype=np.float32)
    gwT = np.ascontiguousarray(gw.T)
    mg = np.asarray(merge_groups).astype(np.int64)
    de = np.asarray(dominant_experts).astype(np.int64)
    gup = np.asarray(gate_up_proj, dtype=np.float32)
    dnp_ = np.asarray(down_proj, dtype=np.float32)

    in_maps = []
    for g in range(G):
        e = int(de[g])
        guT = np.ascontiguousarray(gup[e].T[:, _GATEUP_PERM])
        dnT = np.ascontiguousarray(dnp_[e].T)
        mgb = np.ascontiguousarray(
            np.broadcast_to((mg == g).astype(np.float32)[None, :], (128, E))
        )
        in_maps.append({"xT": xT, "gu": guT, "gw": gwT, "dnT": dnT,
                        "mgb": mgb})
    return in_maps


def kernel(hidden_states, gate_weight, gate_up_proj, down_proj,
           merge_groups, dominant_experts):
    in_maps = prepare_in_maps(hidden_states, gate_weight, gate_up_proj,
                              down_proj, merge_groups, dominant_experts)
    nc = _build()
    res = run_bass_kernel_spmd(nc, in_maps, core_ids=list(range(G)),
                               trace=False)
    out = np.zeros((T, H), dtype=np.float64)
    for r in res.results:
        out += r["y"].astype(np.float64)
    return out.astype(np.float32).reshape(1, T, H)



# revision 3
# speedup vs baseline: 1.8955x; 1.8955x over previous
"""TRN2 Bass kernel for nn_HCSMoEQwen3MoeSparseMoeBlock (8-core, balanced).

Routing is computed on the host (numpy, f64) as part of input sharding:
for each token the top-8 expert probabilities are merged per group into
w[t, g]; only (token, group) pairs with w > 0 are real work (~10.5k of
16.4k here).  That work is balanced across the 8 cores in 128-token
chunks: each core runs NCHUNK chunks split into NSEG=3 segments; each
segment is bound to one (group -> dominant expert) whose gu/dn weights
are streamed into a double-buffered SBUF slot while the previous
segment computes.  Device does, per chunk:
  M1   h = x @ guT    (bf16, xT-chunk stationary, gu moving N=512)
  SwiGLU (scalar silu + DVE mult, host-interleaved [256 gate|256 up])
  PE transpose act -> actT (bf16, 1 cyc/row)
  M2   y = act @ dnT  (bf16, actT stationary, dn moving N=512)
  y *= w[token]  (per-partition scalar), DMA out (f32)
Host scatter-adds the per-slot rows into the full [2048, 2048] output.
"""
import math
from collections import Counter
from itertools import product as _iproduct

import ml_dtypes
import numpy as np

import concourse.bass as bass
import concourse.mybir as mybir
import concourse.tile as tile
from concourse import bacc
from concourse.bass_utils import run_bass_kernel_spmd
from concourse.masks import make_identity

T = 2048
H = 2048
I2 = 1536
I = 768
E = 32
G = 8
TOP_K = 8
KO = H // 128
JO = I // 128
TCH = 128
HB = 512
NSEG = 3

F32 = mybir.dt.float32
BF16 = mybir.dt.bfloat16
AX = mybir.AxisListType.X
OP = mybir.AluOpType
ACTF = mybir.ActivationFunctionType
BF16NP = ml_dtypes.bfloat16

_CACHED_NC = {}


def _build(nchunk, seglens):
    key = (nchunk, tuple(seglens))
    if key in _CACHED_NC:
        return _CACHED_NC[key]
    nc = bacc.Bacc("TRN2", target_bir_lowering=False, debug=False, num_devices=G)

    xT_d = nc.dram_tensor("xT", [H, nchunk * TCH], BF16, kind="ExternalInput")
    gu_d = [nc.dram_tensor(f"gu{s}", [H, I2], BF16, kind="ExternalInput")
            for s in range(NSEG)]
    dn_d = [nc.dram_tensor(f"dn{s}", [I, H], BF16, kind="ExternalInput")
            for s in range(NSEG)]
    wtok_d = nc.dram_tensor("wtok", [TCH, nchunk], F32, kind="ExternalInput")
    y_d = nc.dram_tensor("y", [nchunk * TCH, H], F32, kind="ExternalOutput")

    xT_ap = xT_d.ap().rearrange("(ko p) t -> p ko t", p=128)
    gu_ap = [t.ap().rearrange("(ko p) o -> p ko o", p=128) for t in gu_d]
    dn_ap = [t.ap().rearrange("(jo p) h -> p jo h", p=128) for t in dn_d]

    with tile.TileContext(nc) as tc:
        with (
            tc.tile_pool(name="const", bufs=1) as cpool,
            tc.tile_pool(name="wgu", bufs=2) as gupool,
            tc.tile_pool(name="wdn", bufs=2) as dnpool,
            tc.tile_pool(name="xin", bufs=3) as xpool,
            tc.tile_pool(name="acts", bufs=2) as apool,
            tc.tile_pool(name="yout", bufs=3) as ypool,
            tc.tile_pool(name="ph", bufs=3, space="PSUM") as pph,
            tc.tile_pool(name="pt", bufs=2, space="PSUM") as ppt,
            tc.tile_pool(name="py", bufs=2, space="PSUM") as ppy,
        ):
            identity = cpool.tile([128, 128], BF16, tag="identity")
            make_identity(nc, identity)
            w_sb = cpool.tile([TCH, nchunk], F32, tag="wtok")
            nc.sync.dma_start(w_sb[:], wtok_d.ap())

            gu_tiles = {}
            dn_tiles = {}
            xtiles = {}

            def load_x(ci):
                t = xpool.tile([128, KO, TCH], BF16, tag="xT_c",
                               name=f"xT_c{ci}")
                nc.sync.dma_start(t[:], xT_ap[:, :, ci * TCH:(ci + 1) * TCH])
                xtiles[ci] = t

            def alloc_seg(s):
                g = gupool.tile([128, KO, I2], BF16, tag="gu", name=f"gu{s}")
                d = dnpool.tile([128, JO, H], BF16, tag="dn", name=f"dn{s}")
                gu_tiles[s] = g
                dn_tiles[s] = d
                return g, d

            def seg_load_pieces(s):
                """Return list of DMA thunks loading segment s's weights."""
                g, d = alloc_seg(s)
                th = []
                for b in range(3):
                    th.append(lambda b=b: nc.sync.dma_start(
                        g[:, :, b * HB:(b + 1) * HB],
                        gu_ap[s][:, :, b * HB:(b + 1) * HB]))
                for j0 in range(0, JO, 3):
                    th.append(lambda j0=j0: nc.sync.dma_start(
                        d[:, j0:j0 + 3], dn_ap[s][:, j0:j0 + 3]))
                return th

            # ---- head: seg0 pieces interleaved with x prefetch ----
            load_x(0)
            p0 = seg_load_pieces(0)
            p0[0]()
            load_x(1)
            p0[1]()
            p0[2]()
            p0[3]()
            p0[4]()
            load_x(2)
            # seg1 whole (streams during seg0 compute)
            p1 = seg_load_pieces(1)
            for t in p1:
                t()
            p2 = None
            p2_idx = 0

            ci = 0
            for s in range(NSEG):
                gu_sb = gu_tiles[s]
                dn_sb = dn_tiles[s]
                for cc in range(seglens[s]):
                    if ci + 3 < nchunk:
                        load_x(ci + 3)
                    if s == 1 and cc == 0:
                        p2 = seg_load_pieces(2)
                    if p2 is not None and p2_idx < len(p2):
                        # spread seg2 loads across seg1's chunks
                        n_per = -(-len(p2) // seglens[1])
                        for _ in range(n_per):
                            if p2_idx < len(p2):
                                p2[p2_idx]()
                                p2_idx += 1
                    xT_c = xtiles.pop(ci)

                    # ---- M1 + SwiGLU (b-major, one PSUM bank each) ----
                    act_sb = apool.tile([128, I], BF16, tag="act",
                                        name=f"act{ci}")
                    for b in range(3):
                        h_ps = pph.tile([128, HB], F32, tag="h_ps",
                                        name=f"h{ci}_{b}")
                        for k in range(KO):
                            nc.tensor.matmul(
                                h_ps[:], xT_c[:, k],
                                gu_sb[:, k, b * HB:(b + 1) * HB],
                                start=(k == 0), stop=(k == KO - 1),
                            )
                        silu_sb = apool.tile([128, 256], F32, tag="silu",
                                             name=f"silu{ci}_{b}")
                        nc.scalar.activation(silu_sb[:], h_ps[:, :256],
                                             ACTF.Silu)
                        nc.vector.tensor_tensor(
                            act_sb[:, 256 * b:256 * (b + 1)], silu_sb[:],
                            h_ps[:, 256:], OP.mult,
                        )

                    # ---- transpose act -> actT (bf16, 1 cyc/row) ----
                    actT_sb = apool.tile([128, JO, TCH], BF16, tag="actT",
                                         name=f"actT{ci}")
                    for j in range(JO):
                        tp = ppt.tile([128, TCH], BF16, tag="tp",
                                      name=f"tp{ci}_{j}")
                        nc.tensor.transpose(
                            tp[:], act_sb[:, j * 128:(j + 1) * 128], identity)
                        nc.vector.tensor_copy(actT_sb[:, j], tp[:])

                    # ---- M2 + scale + store ----
                    for hb in range(H // HB):
                        y_ps = ppy.tile([128, HB], F32, tag="y_ps",
                                        name=f"y{ci}_{hb}")
                        for j in range(JO):
                            nc.tensor.matmul(
                                y_ps[:], actT_sb[:, j],
                                dn_sb[:, j, hb * HB:(hb + 1) * HB],
                                start=(j == 0), stop=(j == JO - 1),
                            )
                        y_sb = ypool.tile([128, HB], F32, tag="y_sb",
                                          name=f"ysb{ci}_{hb}")
                        nc.vector.tensor_scalar(
                            y_sb[:], y_ps[:], w_sb[:, ci:ci + 1], None,
                            OP.mult,
                        )
                        nc.sync.dma_start(
                            y_d.ap()[ci * TCH:(ci + 1) * TCH,
                                     hb * HB:(hb + 1) * HB], y_sb[:],
                        )
                    ci += 1
    nc.compile()
    _CACHED_NC[key] = nc
    return nc


_GATEUP_PERM = np.concatenate(
    [np.r_[256 * b:256 * b + 256, 768 + 256 * b:768 + 256 * b + 256]
     for b in range(3)]
)


def _pack(chunks):
    """Assign per-group chunk counts to 8 cores x NSEG fixed-length
    segments.  Returns (nchunk, seglens, per_seg) where per_seg[s] is the
    length-8 list of group ids (-1 = dummy) for segment s across cores."""
    total = sum(chunks)
    lo = max(NSEG, math.ceil(total / 8)) if total else NSEG
    for nchunk in range(lo, lo + 6):
        base, rem = divmod(nchunk, NSEG)
        seglens = [base + 1] * rem + [base] * (NSEG - rem)
        capc = Counter(seglens)
        vals = sorted(capc, reverse=True)
        avail = {v: 8 * capc[v] for v in vals}
        order = sorted(range(G), key=lambda g: -chunks[g])
        assign = {}

        def dfs(gi):
            if gi == len(order):
                return True
            g = order[gi]
            need = chunks[g]
            if need == 0:
                assign[g] = Counter()
                return dfs(gi + 1)
            combos = []
            for ks in _iproduct(*[range(avail[v] + 1) for v in vals]):
                tot = sum(k * v for k, v in zip(ks, vals))
                if tot >= need and tot - need < min(
                        v for k, v in zip(ks, vals) if k):
                    combos.append((tot - need, sum(ks), ks))
            combos.sort()
            for _, _, ks in combos:
                ok = all(avail[v] >= k for k, v in zip(ks, vals))
                if not ok:
                    continue
                for k, v in zip(ks, vals):
                    avail[v] -= k
                assign[g] = Counter(
                    {v: k for k, v in zip(ks, vals) if k})
                if dfs(gi + 1):
                    return True
                for k, v in zip(ks, vals):
                    avail[v] += k
            return False

        if dfs(0):
            seg_entries = {v: [] for v in vals}
            for g in range(G):
                for v, k in assign.get(g, Counter()).items():
                    seg_entries[v].extend([g] * k)
            per_seg = []
            offs = {v: 0 for v in vals}
            for L in seglens:
                lst = seg_entries[L][offs[L]:offs[L] + 8]
                offs[L] += 8
                lst = lst + [-1] * (8 - len(lst))
                per_seg.append(lst)
            return nchunk, seglens, per_seg
    raise RuntimeError("segment packing failed")


def _route(hidden_states, gate_weight, merge_groups):
    """Host router: returns w [T, G] f64 (combined weight per token/group)."""
    x = np.asarray(hidden_states, np.float64).reshape(-1, H)
    gw = np.asarray(gate_weight, np.float64)
    mg = np.asarray(merge_groups).astype(np.int64)
    logits = x @ gw.T
    m = logits.max(axis=1, keepdims=True)
    p = np.exp(logits - m)
    p /= p.sum(axis=1, keepdims=True)
    top8 = np.argpartition(-p, TOP_K - 1, axis=1)[:, :TOP_K]
    tv = np.take_along_axis(p, top8, 1)
    tv /= tv.sum(axis=1, keepdims=True)
    w = np.zeros((x.shape[0], G), np.float64)
    np.add.at(w, (np.arange(x.shape[0])[:, None], mg[top8]), tv)
    return w


def prepare(hidden_states, gate_weight, gate_up_proj, down_proj,
            merge_groups, dominant_experts):
    w = _route(hidden_states, gate_weight, merge_groups)
    de = np.asarray(dominant_experts).astype(np.int64)
    ids = [np.nonzero(w[:, g] > 0)[0] for g in range(G)]
    chunks = [-(-len(i) // TCH) if len(i) else 0 for i in ids]
    nchunk, seglens, per_seg = _pack(chunks)

    x32 = np.asarray(hidden_states, np.float32).reshape(-1, H)
    gup = np.asarray(gate_up_proj, np.float32)
    dnp_ = np.asarray(down_proj, np.float32)

    # per-expert weight tensors (bf16), computed once per unique expert
    guT_cache = {}
    dnT_cache = {}
    for g in range(G):
        e = int(de[g])
        if e not in guT_cache:
            guT_cache[e] = np.ascontiguousarray(
                gup[e].T[:, _GATEUP_PERM]).astype(BF16NP)
            dnT_cache[e] = np.ascontiguousarray(dnp_[e].T).astype(BF16NP)
    gu_zero = np.zeros((H, I2), BF16NP)
    dn_zero = np.zeros((I, H), BF16NP)

    # distribute each group's tokens over its slots in (seg, core) order
    consumed = [0] * G
    slots = []  # records: (core, seg, chunk_start, n_real, token_ids)
    tok_full = [np.zeros(nchunk * TCH, np.int64) for _ in range(8)]
    w_full = [np.zeros(nchunk * TCH, np.float32) for _ in range(8)]
    seg_start = [sum(seglens[:s]) for s in range(NSEG)]
    core_seg_group = [[-1] * NSEG for _ in range(8)]
    for s in range(NSEG):
        for c in range(8):
            g = per_seg[s][c]
            core_seg_group[c][s] = g
            if g < 0:
                continue
            cap = seglens[s] * TCH
            take = min(cap, len(ids[g]) - consumed[g])
            if take <= 0:
                continue
            tk = ids[g][consumed[g]:consumed[g] + take]
            consumed[g] += take
            off = seg_start[s] * TCH
            tok_full[c][off:off + take] = tk
            w_full[c][off:off + take] = w[tk, g].astype(np.float32)
            slots.append((c, off, take, tk))
    for g in range(G):
        assert consumed[g] == len(ids[g]), "token assignment incomplete"

    in_maps = []
    for c in range(8):
        xT = np.ascontiguousarray(x32[tok_full[c]].T).astype(BF16NP)
        wmat = np.ascontiguousarray(
            w_full[c].reshape(nchunk, TCH).T)
        im = {"xT": xT, "wtok": wmat}
        for s in range(NSEG):
            g = core_seg_group[c][s]
            if g < 0:
                im[f"gu{s}"] = gu_zero
                im[f"dn{s}"] = dn_zero
            else:
                e = int(de[g])
                im[f"gu{s}"] = guT_cache[e]
                im[f"dn{s}"] = dnT_cache[e]
        in_maps.append(im)
    return nchunk, seglens, in_maps, slots


def kernel(hidden_states, gate_weight, gate_up_proj, down_proj,
           merge_groups, dominant_experts):
    nchunk, seglens, in_maps, slots = prepare(
        hidden_states, gate_weight, gate_up_proj, down_proj,
        merge_groups, dominant_experts)
    nc = _build(nchunk, seglens)
    res = run_bass_kernel_spmd(nc, in_maps, core_ids=list(range(8)),
                               trace=False)
    out = np.zeros((T, H), np.float64)
    for c, off, take, tk in slots:
        out[tk] += res.results[c]["y"][off:off + take].astype(np.float64)
    return out.astype(np.float32).reshape(1, T, H)
